# revision 22
# baseline (speedup 1.0000x reference)
"""Trainium2 Bass kernel for nn_AcceptHead: out = fc2(gelu(fc1(LN(x)))).

Self-contained: hardcodes shapes (B=4, L=4096, H=4096, F=1024) and the
data-parallel sharding (tokens split 8 ways, head params replicated).

Per-core dataflow (2048 tokens each):
  - LayerNorm in natural layout [128 tok, 4096 h]:
      mean: DVE reduce_sum; sumsq: DVE fused multiply+reduce;
      rstd: Newton rsqrt on DVE (int bit-trick seed, 3 iterations);
      normalize+cast fp32->fp16 fused on ScalarE (per-partition scale/bias).
  - Transpose [tok, h] -> [h, tok] via xbar DMA transpose (fp16).
  - fc1 on TensorE fp16: lhsT = xT block [128h x 128t], rhs = w1T [128h x 512f],
    PSUM-accumulate over 32 k-tiles -> [128 tok, 1024 f] fp32.
  - bias1 (if nonzero) + exact GELU on ScalarE, PSUM -> SBUF fp16.
  - fc2 as fused DVE dot: (g * w2_bcast) summed over the free dim.
  - gamma/beta folded into w1/bias1 on host (exact); biases skipped if zero.
"""

import os
import sys

for _p in ("/opt/trn_rl_repo", "/root/.axon_site/_ro/trn_rl_repo"):
    if os.path.isdir(_p) and _p not in sys.path:
        sys.path.append(_p)

import numpy as np

import concourse.bacc as bacc
import concourse.mybir as mybir
import concourse.tile as tile
from concourse.bass_utils import run_bass_kernel_spmd

N_CORES = 8
B, L, H = 4, 4096, 4096
F = H // 4
T_TOT = B * L                 # 16384 tokens
T_CORE = T_TOT // N_CORES     # 2048 tokens per core
P = 128
KT = H // P                   # 32 contraction tiles
CHUNK_T = 256                 # tokens per pipeline chunk
N_CHUNKS = T_CORE // CHUNK_T  # 8
TT = CHUNK_T // P             # t-tiles per chunk (2)
N_TTILES = T_CORE // P        # 16
EPS = 1e-5
RSQRT_MAGIC = 0x5F3759DF

F16 = mybir.dt.float16
F32 = mybir.dt.float32
I32 = mybir.dt.int32
AF = mybir.ActivationFunctionType
ALU = mybir.AluOpType


def build_program(has_bias1: bool, bias2_val: float):
    nc = bacc.Bacc(
        "TRN2",
        target_bir_lowering=False,
        debug=False,
        enable_asserts=False,
        num_devices=N_CORES,
    )
    x_d = nc.dram_tensor("x", [T_CORE, H], F32, kind="ExternalInput").ap()
    w1t_d = nc.dram_tensor("w1t", [H, F], F16, kind="ExternalInput").ap()
    w2b_d = nc.dram_tensor("w2b", [P, F], F16, kind="ExternalInput").ap()
    if has_bias1:
        b1b_d = nc.dram_tensor("b1b", [P, F], F32, kind="ExternalInput").ap()
    out_d = nc.dram_tensor("out", [T_CORE], F32, kind="ExternalOutput").ap()

    with tile.TileContext(nc) as tc:
        with (
            tc.tile_pool(name="singles", bufs=1) as singles,
            tc.tile_pool(name="xpool", bufs=4) as xpool,
            tc.tile_pool(name="xnpool", bufs=3) as xnpool,
            tc.tile_pool(name="xtpool", bufs=2) as xtpool,
            tc.tile_pool(name="sqscr", bufs=1) as sqscr_pool,
            tc.tile_pool(name="gpool", bufs=2) as gpool,
            tc.tile_pool(name="fc2scr", bufs=1) as fc2scr_pool,
            tc.tile_pool(name="stats", bufs=4) as stats,
            tc.tile_pool(name="psum", bufs=2, space="PSUM") as psum_pool,
            tc.tile_pool(name="tpsum", bufs=2, space="PSUM") as tpsum_pool,
        ):
            # Loads go on the ACT HWDGE ring (nc.scalar), transposes on the SP
            # ring (nc.sync): HWDGE DMAs are FIFO per issuing engine, so mixing
            # them would head-of-line-block next-chunk loads behind transposes
            # that wait on this chunk's normalize.
            def load_chunk(c):
                tiles = []
                for i in range(TT):
                    xt = xpool.tile([P, H], F32, tag="x")
                    row0 = (c * TT + i) * P
                    nc.sync.dma_start(out=xt, in_=x_d[row0 : row0 + P, :])
                    tiles.append(xt)
                return tiles

            # Variable chunk sizes: two single-tile chunks first so the
            # pipeline (stats -> rstd -> normalize -> transpose -> matmul)
            # fills fast, then steady-state 2-tile chunks.
            chunks = [1] * 4 + [2] * ((N_TTILES - 4) // 2)
            starts = [sum(chunks[:i]) for i in range(len(chunks))]

            def load_tiles(c):
                tiles = []
                for i in range(chunks[c]):
                    xt = xpool.tile([P, H], F32, tag="x")
                    row0 = (starts[c] + i) * P
                    nc.sync.dma_start(out=xt, in_=x_d[row0 : row0 + P, :])
                    tiles.append(xt)
                return tiles

            x_cur = load_tiles(0)
            x_next = load_tiles(1)

            # ---- persistent tiles (w1t in k-quarters so early MMs don't wait
            # on the whole 8MB load) ----
            QK = 4
            w1t_sb = []
            w1t_r = w1t_d.rearrange("(q k p) f -> q p k f", q=QK, p=P)
            for q in range(QK):
                wq = singles.tile([P, KT // QK, F], F16, tag=f"w1t{q}")
                nc.gpsimd.dma_start(out=wq, in_=w1t_r[q])
                w1t_sb.append(wq)
            w2b_sb = singles.tile([P, F], F16)
            nc.gpsimd.dma_start(out=w2b_sb, in_=w2b_d)
            if has_bias1:
                b1b_sb = singles.tile([P, F], F32)
                nc.gpsimd.dma_start(out=b1b_sb, in_=b1b_d)
            outcols = singles.tile([P, N_TTILES], F32)
            ident = singles.tile([P, P], F16)
            from concourse.masks import make_identity
            make_identity(nc, ident[:])


            for c in range(len(chunks)):
                x_tiles = x_cur
                x_cur = x_next
                ntt = chunks[c]
                # ---- stats per x tile ----
                sums = stats.tile([P, TT], F32, tag="sums")
                sq = stats.tile([P, TT], F32, tag="sq")
                for i in range(ntt):
                    xt = x_tiles[i]
                    nc.vector.reduce_sum(
                        sums[:, i : i + 1], xt, axis=mybir.AxisListType.X
                    )
                    sqs = sqscr_pool.tile([P, H], F16, tag="sqs")
                    nc.scalar.activation(
                        out=sqs, in_=xt, func=AF.Square, accum_out=sq[:, i : i + 1]
                    )

                # ---- per-chunk scalar math on [P, ntt]: var, rstd, -mu*rstd ----
                sums = sums[:, :ntt]
                sq = sq[:, :ntt]
                mu_t = stats.tile([P, TT], F32, tag="mu")
                mu = mu_t[:, :ntt]
                nc.vector.tensor_scalar_mul(mu, sums, 1.0 / H)
                vv_t = stats.tile([P, TT], F32, tag="vv")
                vv = vv_t[:, :ntt]
                # vv = sq/H - mu^2 + eps
                nc.vector.tensor_tensor(out=vv, in0=mu, in1=mu, op=ALU.mult)
                nc.vector.tensor_scalar(
                    out=vv, in0=vv, scalar1=-1.0, scalar2=EPS, op0=ALU.mult, op1=ALU.add
                )
                nc.vector.tensor_scalar(
                    out=sq, in0=sq, scalar1=1.0 / H, scalar2=None, op0=ALU.mult
                )
                nc.vector.tensor_tensor(out=vv, in0=vv, in1=sq, op=ALU.add)
                # Newton rsqrt: y0 via bit trick, 3 iterations
                y_t = stats.tile([P, TT], F32, tag="y")
                y = y_t[:, :ntt]
                yi = y[:].bitcast(I32)
                nc.vector.tensor_scalar(
                    out=yi, in0=vv[:].bitcast(I32), scalar1=1, scalar2=None,
                    op0=ALU.arith_shift_right,
                )
                nc.vector.tensor_scalar(
                    out=yi, in0=yi, scalar1=-1, scalar2=RSQRT_MAGIC,
                    op0=ALU.mult, op1=ALU.add,
                )
                h_half_t = stats.tile([P, TT], F32, tag="h_half")
                h_half = h_half_t[:, :ntt]
                nc.vector.tensor_scalar_mul(h_half, vv, 0.5)
                u_t = stats.tile([P, TT], F32, tag="u")
                u = u_t[:, :ntt]
                for _ in range(3):
                    nc.vector.tensor_tensor(out=u, in0=y, in1=y, op=ALU.mult)
                    nc.vector.tensor_tensor(out=u, in0=u, in1=h_half, op=ALU.mult)
                    nc.vector.tensor_scalar(
                        out=u, in0=u, scalar1=-1.0, scalar2=1.5,
                        op0=ALU.mult, op1=ALU.add,
                    )
                    nc.vector.tensor_tensor(out=y, in0=y, in1=u, op=ALU.mult)
                # nmr = -mu * rstd
                nmr_t = stats.tile([P, TT], F32, tag="nmr")
                nmr = nmr_t[:, :ntt]
                nc.vector.tensor_tensor(out=nmr, in0=mu, in1=y, op=ALU.mult)
                nc.vector.tensor_scalar_mul(nmr, nmr, -1.0)

                # ---- normalize (fp32 -> fp16) + transpose ----
                # First two chunks transpose on the PE (+PSUM->SBUF copy):
                # the xbar path would stall on mode-quiesce against the big
                # in-flight w1t copy; PE transposes keep the early pipeline
                # dense. Steady state uses the cheap xbar path.
                xT = xtpool.tile([P, KT, CHUNK_T], F16)
                for i in range(ntt):
                    xn = xnpool.tile([P, H], F16)
                    nc.scalar.activation(
                        out=xn,
                        in_=x_tiles[i],
                        func=AF.Identity,
                        bias=nmr[:, i : i + 1],
                        scale=y[:, i : i + 1],
                    )
                    if c < 4:
                        for kb in range(4):
                            tps = tpsum_pool.tile([P, 8, P], F16, tag="tps")
                            for kk in range(8):
                                k = kb * 8 + kk
                                nc.tensor.transpose(
                                    tps[:, kk, :],
                                    xn[:, k * P : (k + 1) * P],
                                    ident[:],
                                )
                            nc.vector.tensor_copy(
                                out=xT[:, kb * 8 : (kb + 1) * 8,
                                       i * P : (i + 1) * P],
                                in_=tps,
                            )
                    else:
                        nc.sync.dma_start_transpose(
                            xT[:, :, i * P : (i + 1) * P], xn
                        )
                # next-chunk loads go on the SP ring AFTER this chunk's
                # transposes: the xbar-mode switch serializes a transpose
                # against in-flight copies, so copies emitted first would
                # head-of-line-block the transposes.
                if c + 2 < len(chunks):
                    x_next = load_tiles(c + 2)

                # ---- fc1 + gelu + fc2 per t-tile ----
                for i in range(ntt):
                    g_ps = psum_pool.tile([P, F], F32, tag="g_ps")
                    for fh in range(2):
                        fcols = slice(fh * 512, (fh + 1) * 512)
                        for k in range(KT):
                            nc.tensor.matmul(
                                g_ps[:, fcols],
                                lhsT=xT[:, k, i * P : (i + 1) * P],
                                rhs=w1t_sb[k // (KT // 4)][:, k % (KT // 4), fcols],
                                start=(k == 0),
                                stop=(k == KT - 1),
                            )
                    if has_bias1:
                        nc.vector.tensor_tensor(
                            out=g_ps, in0=g_ps, in1=b1b_sb, op=ALU.add
                        )
                    g_sb = gpool.tile([P, F], F16, tag="g_sb")
                    nc.scalar.activation(out=g_sb, in_=g_ps, func=AF.Gelu)
                    fc2s = fc2scr_pool.tile([P, F], F16, tag="fc2s")
                    gi = starts[c] + i
                    nc.vector.tensor_tensor(
                        out=fc2s, in0=g_sb, in1=w2b_sb, op=ALU.mult
                    )
                    nc.vector.reduce_sum(
                        outcols[:, gi : gi + 1], fc2s, axis=mybir.AxisListType.X
                    )

            if bias2_val != 0.0:
                nc.vector.tensor_scalar_add(outcols, outcols, bias2_val)
            nc.sync.dma_start(
                out=out_d.rearrange("(n p) -> p n", p=P), in_=outcols
            )

    nc.compile()
    return nc


def _prep_host(hidden_states, ln_gamma, ln_beta, w1, bias1, w2, bias2):
    """Exact host-side folding of LN affine params into fc1 (all in float64)."""
    g64 = np.asarray(ln_gamma, np.float64)
    b64 = np.asarray(ln_beta, np.float64)
    w1_64 = np.asarray(w1, np.float64)
    w1t = np.ascontiguousarray((w1_64 * g64[None, :]).T).astype(np.float16)
    b1_eff = (np.asarray(bias1, np.float64) + w1_64 @ b64).astype(np.float32)
    w2b = np.broadcast_to(
        np.asarray(w2, np.float64).reshape(1, F).astype(np.float16), (P, F)
    ).copy()
    bias2_val = float(np.asarray(bias2).reshape(-1)[0])
    x2 = np.ascontiguousarray(np.asarray(hidden_states, np.float32).reshape(T_TOT, H))
    return x2, w1t, b1_eff, w2b, bias2_val


_CACHE = {}


def _get_program(has_bias1, bias2_val):
    key = (has_bias1, bias2_val)
    if key not in _CACHE:
        _CACHE[key] = build_program(has_bias1, bias2_val)
    return _CACHE[key]


def make_in_maps(inputs):
    x2, w1t, b1_eff, w2b, bias2_val = _prep_host(**inputs)
    has_bias1 = bool(np.any(b1_eff != 0.0))
    in_maps = []
    for core in range(N_CORES):
        m = {
            "x": np.ascontiguousarray(x2[core * T_CORE : (core + 1) * T_CORE]),
            "w1t": w1t,
            "w2b": w2b,
        }
        if has_bias1:
            m["b1b"] = np.broadcast_to(b1_eff[None, :], (P, F)).copy()
        in_maps.append(m)
    return in_maps, has_bias1, bias2_val


def kernel(**inputs) -> np.ndarray:
    in_maps, has_bias1, bias2_val = make_in_maps(inputs)
    nc = _get_program(has_bias1, bias2_val)
    res = run_bass_kernel_spmd(nc, in_maps, core_ids=list(range(N_CORES)))
    out = np.concatenate([res.results[i]["out"] for i in range(N_CORES)])
    return out.reshape(B, L).astype(np.float32)


# revision 23
# speedup vs baseline: 1.1366x; 1.1366x over previous
"""Trainium2 Bass kernel for nn_AcceptHead: out = fc2(gelu(fc1(LN(x)))).

Self-contained: hardcodes shapes (B=4, L=4096, H=4096, F=1024) and the
data-parallel sharding (tokens split 8 ways, head params replicated).

Per-core dataflow (2048 tokens each):
  - LayerNorm in natural layout [128 tok, 4096 h]:
      mean: DVE reduce_sum; sumsq: DVE fused multiply+reduce;
      rstd: Newton rsqrt on DVE (int bit-trick seed, 3 iterations);
      normalize+cast fp32->fp16 fused on ScalarE (per-partition scale/bias).
  - Transpose [tok, h] -> [h, tok] via xbar DMA transpose (fp16).
  - fc1 on TensorE fp16: lhsT = xT block [128h x 128t], rhs = w1T [128h x 512f],
    PSUM-accumulate over 32 k-tiles -> [128 tok, 1024 f] fp32.
  - bias1 (if nonzero) + exact GELU on ScalarE, PSUM -> SBUF fp16.
  - fc2 as fused DVE dot: (g * w2_bcast) summed over the free dim.
  - gamma/beta folded into w1/bias1 on host (exact); biases skipped if zero.
"""

import os
import sys

for _p in ("/opt/trn_rl_repo", "/root/.axon_site/_ro/trn_rl_repo"):
    if os.path.isdir(_p) and _p not in sys.path:
        sys.path.append(_p)

import numpy as np

import concourse.bacc as bacc
import concourse.mybir as mybir
import concourse.tile as tile
from concourse.bass_utils import run_bass_kernel_spmd

N_CORES = 8
B, L, H = 4, 4096, 4096
F = H // 4
T_TOT = B * L                 # 16384 tokens
T_CORE = T_TOT // N_CORES     # 2048 tokens per core
P = 128
KT = H // P                   # 32 contraction tiles
CHUNK_T = 256                 # tokens per pipeline chunk
N_CHUNKS = T_CORE // CHUNK_T  # 8
TT = CHUNK_T // P             # t-tiles per chunk (2)
N_TTILES = T_CORE // P        # 16
EPS = 1e-5
RSQRT_MAGIC = 0x5F3759DF

F16 = mybir.dt.float16
F32 = mybir.dt.float32
I32 = mybir.dt.int32
AF = mybir.ActivationFunctionType
ALU = mybir.AluOpType


def build_program(has_bias1: bool, bias2_val: float):
    nc = bacc.Bacc(
        "TRN2",
        target_bir_lowering=False,
        debug=False,
        enable_asserts=False,
        num_devices=N_CORES,
    )
    x_d = nc.dram_tensor("x", [T_CORE, H], F32, kind="ExternalInput").ap()
    w1t_d = nc.dram_tensor("w1t", [H, F], F16, kind="ExternalInput").ap()
    w2b_d = nc.dram_tensor("w2b", [P, F], F16, kind="ExternalInput").ap()
    if has_bias1:
        b1b_d = nc.dram_tensor("b1b", [P, F], F32, kind="ExternalInput").ap()
    out_d = nc.dram_tensor("out", [T_CORE], F32, kind="ExternalOutput").ap()

    with tile.TileContext(nc) as tc:
        with (
            tc.tile_pool(name="singles", bufs=1) as singles,
            tc.tile_pool(name="xpool", bufs=4) as xpool,
            tc.tile_pool(name="xnpool", bufs=3) as xnpool,
            tc.tile_pool(name="xtpool", bufs=2) as xtpool,
            tc.tile_pool(name="sqscr", bufs=1) as sqscr_pool,
            tc.tile_pool(name="gpool", bufs=2) as gpool,
            tc.tile_pool(name="fc2scr", bufs=1) as fc2scr_pool,
            tc.tile_pool(name="stats", bufs=4) as stats,
            tc.tile_pool(name="psum", bufs=3, space="PSUM") as psum_pool,
        ):
            # Variable chunk sizes: two single-tile chunks first so the
            # pipeline (stats -> rstd -> normalize -> transpose -> matmul)
            # fills fast, then steady-state 2-tile chunks.
            chunks = [2] * (N_TTILES // 2)
            starts = [sum(chunks[:i]) for i in range(len(chunks))]

            def load_tiles(c):
                tiles = []
                for i in range(chunks[c]):
                    xt = xpool.tile([P, H], F32, tag="x")
                    row0 = (starts[c] + i) * P
                    nc.sync.dma_start(out=xt, in_=x_d[row0 : row0 + P, :])
                    tiles.append(xt)
                return tiles

            x_cur = load_tiles(0)
            x_next = load_tiles(1)

            # ---- persistent tiles (w1t in k-quarters so early MMs don't wait
            # on the whole 8MB load) ----
            QK = 4
            w1t_sb = []
            w1t_r = w1t_d.rearrange("(q k p) f -> q p k f", q=QK, p=P)
            for q in range(QK):
                wq = singles.tile([P, KT // QK, F], F16, tag=f"w1t{q}")
                nc.gpsimd.dma_start(out=wq, in_=w1t_r[q])
                w1t_sb.append(wq)
            w2b_sb = singles.tile([P, F], F16)
            nc.gpsimd.dma_start(out=w2b_sb, in_=w2b_d)
            if has_bias1:
                b1b_sb = singles.tile([P, F], F32)
                nc.gpsimd.dma_start(out=b1b_sb, in_=b1b_d)
            outcols = singles.tile([P, N_TTILES], F32)



            for c in range(len(chunks)):
                x_tiles = x_cur
                x_cur = x_next
                ntt = chunks[c]
                # ---- stats per x tile ----
                sums = stats.tile([P, TT], F32, tag="sums")
                sq = stats.tile([P, TT], F32, tag="sq")
                for i in range(ntt):
                    xt = x_tiles[i]
                    nc.vector.reduce_sum(
                        sums[:, i : i + 1], xt, axis=mybir.AxisListType.X
                    )
                    sqs = sqscr_pool.tile([P, H], F16, tag="sqs")
                    nc.scalar.activation(
                        out=sqs, in_=xt, func=AF.Square, accum_out=sq[:, i : i + 1]
                    )

                # ---- per-chunk scalar math on [P, ntt]: var, rstd, -mu*rstd ----
                sums = sums[:, :ntt]
                sq = sq[:, :ntt]
                mu_t = stats.tile([P, TT], F32, tag="mu")
                mu = mu_t[:, :ntt]
                nc.vector.tensor_scalar_mul(mu, sums, 1.0 / H)
                vv_t = stats.tile([P, TT], F32, tag="vv")
                vv = vv_t[:, :ntt]
                # vv = sq/H - mu^2 + eps
                nc.vector.tensor_tensor(out=vv, in0=mu, in1=mu, op=ALU.mult)
                nc.vector.tensor_scalar(
                    out=vv, in0=vv, scalar1=-1.0, scalar2=EPS, op0=ALU.mult, op1=ALU.add
                )
                nc.vector.tensor_scalar(
                    out=sq, in0=sq, scalar1=1.0 / H, scalar2=None, op0=ALU.mult
                )
                nc.vector.tensor_tensor(out=vv, in0=vv, in1=sq, op=ALU.add)
                # Newton rsqrt: y0 via bit trick, 3 iterations
                y_t = stats.tile([P, TT], F32, tag="y")
                y = y_t[:, :ntt]
                yi = y[:].bitcast(I32)
                nc.vector.tensor_scalar(
                    out=yi, in0=vv[:].bitcast(I32), scalar1=1, scalar2=None,
                    op0=ALU.arith_shift_right,
                )
                nc.vector.tensor_scalar(
                    out=yi, in0=yi, scalar1=-1, scalar2=RSQRT_MAGIC,
                    op0=ALU.mult, op1=ALU.add,
                )
                h_half_t = stats.tile([P, TT], F32, tag="h_half")
                h_half = h_half_t[:, :ntt]
                nc.vector.tensor_scalar_mul(h_half, vv, 0.5)
                u_t = stats.tile([P, TT], F32, tag="u")
                u = u_t[:, :ntt]
                for _ in range(3):
                    nc.vector.tensor_tensor(out=u, in0=y, in1=y, op=ALU.mult)
                    nc.vector.tensor_tensor(out=u, in0=u, in1=h_half, op=ALU.mult)
                    nc.vector.tensor_scalar(
                        out=u, in0=u, scalar1=-1.0, scalar2=1.5,
                        op0=ALU.mult, op1=ALU.add,
                    )
                    nc.vector.tensor_tensor(out=y, in0=y, in1=u, op=ALU.mult)
                # nmr = -mu * rstd
                nmr_t = stats.tile([P, TT], F32, tag="nmr")
                nmr = nmr_t[:, :ntt]
                nc.vector.tensor_tensor(out=nmr, in0=mu, in1=y, op=ALU.mult)
                nc.vector.tensor_scalar_mul(nmr, nmr, -1.0)

                # ---- normalize (fp32 -> fp16) + transpose ----
                # First two chunks transpose on the PE (+PSUM->SBUF copy):
                # the xbar path would stall on mode-quiesce against the big
                # in-flight w1t copy; PE transposes keep the early pipeline
                # dense. Steady state uses the cheap xbar path.
                xT = xtpool.tile([P, KT, CHUNK_T], F16)
                for i in range(ntt):
                    xn = xnpool.tile([P, H], F16)
                    nc.scalar.activation(
                        out=xn,
                        in_=x_tiles[i],
                        func=AF.Identity,
                        bias=nmr[:, i : i + 1],
                        scale=y[:, i : i + 1],
                    )
                    nc.sync.dma_start_transpose(
                        xT[:, :, i * P : (i + 1) * P], xn
                    )
                # next-chunk loads go on the SP ring AFTER this chunk's
                # transposes: the xbar-mode switch serializes a transpose
                # against in-flight copies, so copies emitted first would
                # head-of-line-block the transposes.
                if c + 2 < len(chunks):
                    x_next = load_tiles(c + 2)

                # ---- fc1 + gelu + fc2 per t-tile ----
                for i in range(ntt):
                    g_ps = psum_pool.tile([P, F], F32, tag="g_ps")
                    for fh in range(2):
                        fcols = slice(fh * 512, (fh + 1) * 512)
                        for k in range(KT):
                            nc.tensor.matmul(
                                g_ps[:, fcols],
                                lhsT=xT[:, k, i * P : (i + 1) * P],
                                rhs=w1t_sb[k // (KT // 4)][:, k % (KT // 4), fcols],
                                start=(k == 0),
                                stop=(k == KT - 1),
                            )
                    if has_bias1:
                        nc.vector.tensor_tensor(
                            out=g_ps, in0=g_ps, in1=b1b_sb, op=ALU.add
                        )
                    g_sb = gpool.tile([P, F], F16, tag="g_sb")
                    nc.scalar.activation(out=g_sb, in_=g_ps, func=AF.Gelu)
                    fc2s = fc2scr_pool.tile([P, F], F16, tag="fc2s")
                    gi = starts[c] + i
                    nc.vector.tensor_tensor(
                        out=fc2s, in0=g_sb, in1=w2b_sb, op=ALU.mult
                    )
                    nc.vector.reduce_sum(
                        outcols[:, gi : gi + 1], fc2s, axis=mybir.AxisListType.X
                    )

            if bias2_val != 0.0:
                nc.vector.tensor_scalar_add(outcols, outcols, bias2_val)
            nc.sync.dma_start(
                out=out_d.rearrange("(n p) -> p n", p=P), in_=outcols
            )

    nc.compile()
    return nc


def _prep_host(hidden_states, ln_gamma, ln_beta, w1, bias1, w2, bias2):
    """Exact host-side folding of LN affine params into fc1 (all in float64)."""
    g64 = np.asarray(ln_gamma, np.float64)
    b64 = np.asarray(ln_beta, np.float64)
    w1_64 = np.asarray(w1, np.float64)
    w1t = np.ascontiguousarray((w1_64 * g64[None, :]).T).astype(np.float16)
    b1_eff = (np.asarray(bias1, np.float64) + w1_64 @ b64).astype(np.float32)
    w2b = np.broadcast_to(
        np.asarray(w2, np.float64).reshape(1, F).astype(np.float16), (P, F)
    ).copy()
    bias2_val = float(np.asarray(bias2).reshape(-1)[0])
    x2 = np.ascontiguousarray(np.asarray(hidden_states, np.float32).reshape(T_TOT, H))
    return x2, w1t, b1_eff, w2b, bias2_val


_CACHE = {}


def _get_program(has_bias1, bias2_val):
    key = (has_bias1, bias2_val)
    if key not in _CACHE:
        _CACHE[key] = build_program(has_bias1, bias2_val)
    return _CACHE[key]


def make_in_maps(inputs):
    x2, w1t, b1_eff, w2b, bias2_val = _prep_host(**inputs)
    has_bias1 = bool(np.any(b1_eff != 0.0))
    in_maps = []
    for core in range(N_CORES):
        m = {
            "x": np.ascontiguousarray(x2[core * T_CORE : (core + 1) * T_CORE]),
            "w1t": w1t,
            "w2b": w2b,
        }
        if has_bias1:
            m["b1b"] = np.broadcast_to(b1_eff[None, :], (P, F)).copy()
        in_maps.append(m)
    return in_maps, has_bias1, bias2_val


def kernel(**inputs) -> np.ndarray:
    in_maps, has_bias1, bias2_val = make_in_maps(inputs)
    nc = _get_program(has_bias1, bias2_val)
    res = run_bass_kernel_spmd(nc, in_maps, core_ids=list(range(N_CORES)))
    out = np.concatenate([res.results[i]["out"] for i in range(N_CORES)])
    return out.reshape(B, L).astype(np.float32)
